# revision 8
# baseline (speedup 1.0000x reference)
"""GQA with RoPE + sliding-window causal attention on 8 TRN2 NeuronCores.

Sharding: batch (2) x KV-groups (4) -> 8 cores, pure SPMD (no collectives).
Each core computes q/k/v projections for its (batch, group), RoPE, windowed
attention (window=512), and a partial output projection against its group's
WO columns. Host sums the 4 group partials per batch element.

v2: software-pipelined single loop. Per step s, four stages on different
row tiles run concurrently: S0 QKV projection (tile s), S1 scores+exp+mask
(tile s-O1), S2 AV+normalize (tile s-O2), S3 WO partial + output (s-O3).
All matmuls are bf16 (1 cycle/row). The q/k transposes and attn transpose
run on the DMA XBAR (dma_start_transpose), not the PE. x is fully SBUF-
resident (bf16); outputs are written bf16 and summed on host in f32.

qk layout per tile: 3 slabs of 128 partitions x 128 tokens:
  slab0 = [q0|q1] dims, slab1 = [q2|q3], slab2 = [k|k-copy]; scores run
  per-head with 64-partition operands (heads 1,3 at partition base 64
  against the k-copy). Probs accumulate [v|1] so col 64 is the softmax
  denominator. Sliding-window masking multiplies the two edge key blocks
  on GPSIMD off the PE critical path; score chunks are ordered so the
  diagonal (causal) block's exp lands first.
"""

import sys

sys.path.insert(0, "/opt/trn_rl_repo")

import numpy as np
from contextlib import ExitStack

D_MODEL = 1024
GROUP_SIZE = 4
NUM_GROUPS = 4
D_K = 64
THETA = 10000.0
WINDOW = 512
T = 2048
B = 2
NT = T // 128  # 16 row tiles
HALF = D_K // 2

# pipeline offsets: stage S1/S2/S3 of step s handle tiles s-O1/s-O2/s-O3
O1, O2, O3 = 2, 4, 5

_PROGRAM = None


def _build_program():
    from concourse import bacc, tile
    import concourse.mybir as mybir

    f32 = mybir.dt.float32
    bf16 = mybir.dt.bfloat16
    Exp = mybir.ActivationFunctionType.Exp
    mult = mybir.AluOpType.mult
    subtract = mybir.AluOpType.subtract
    add = mybir.AluOpType.add

    nc = bacc.Bacc("TRN2", target_bir_lowering=False, debug=False, num_devices=8)

    xt_d = nc.dram_tensor("xt", [128, NT, 8, 128], bf16, kind="ExternalInput").ap()
    wq_d = nc.dram_tensor("wqkvT", [128, 8, 384], bf16, kind="ExternalInput").ap()
    wo_d = nc.dram_tensor("woT", [128, 2, 1024], bf16, kind="ExternalInput").ap()
    cos_d = nc.dram_tensor("cosT", [128, NT, HALF], f32, kind="ExternalInput").ap()
    sin_d = nc.dram_tensor("sinT", [128, NT, HALF], f32, kind="ExternalInput").ap()
    md_d = nc.dram_tensor("maskd", [128, 1, 128], bf16, kind="ExternalInput").ap()
    mo_d = nc.dram_tensor("masko", [128, 1, 128], bf16, kind="ExternalInput").ap()
    out_d = nc.dram_tensor("out", [128, NT, 1024], bf16, kind="ExternalOutput").ap()

    with tile.TileContext(nc) as tc:
        with ExitStack() as ctx:
            persist = ctx.enter_context(tc.tile_pool(name="persist", bufs=1))
            wq_sb = persist.tile([128, 8, 384], bf16, tag="wq")
            wo_sb = persist.tile([128, 2, 1024], bf16, tag="wo")
            cos_sb = persist.tile([128, NT, HALF], f32, tag="cos")
            sin_sb = persist.tile([128, NT, HALF], f32, tag="sin")
            md_sb = persist.tile([128, 1, 128], bf16, tag="md")
            mo_sb = persist.tile([128, 1, 128], bf16, tag="mo")
            xt_sb = persist.tile([128, NT, 8, 128], bf16, tag="xt")
            qk_sb = persist.tile([128, NT, 3, 128], bf16, tag="qk")
            v_sb = persist.tile([128, NT, 65], bf16, tag="v")

            # startup DMAs: first QKV matmul needs wq + xt tile 0 only
            nc.sync.dma_start(xt_sb[:, 0:1], xt_d[:, 0:1])
            nc.sync.dma_start(wq_sb[:, 0:4], wq_d[:, 0:4])
            nc.sync.dma_start(xt_sb[:, 1:2], xt_d[:, 1:2])
            nc.sync.dma_start(wq_sb[:, 4:8], wq_d[:, 4:8])
            nc.sync.dma_start(cos_sb[:], cos_d[:])
            nc.sync.dma_start(sin_sb[:], sin_d[:])
            nc.sync.dma_start(xt_sb[:, 2:4], xt_d[:, 2:4])
            nc.sync.dma_start(md_sb[:], md_d[:])
            nc.sync.dma_start(mo_sb[:], mo_d[:])
            nc.sync.dma_start(xt_sb[:, 4:6], xt_d[:, 4:6])
            nc.sync.dma_start(wo_sb[:], wo_d[:])
            nc.vector.memset(v_sb[:, :, 64:65], 1.0)

            sc_pool = ctx.enter_context(tc.tile_pool(name="scp", bufs=2, space="PSUM"))
            pp_pool = ctx.enter_context(tc.tile_pool(name="ppp", bufs=1, space="PSUM"))
            av_pool = ctx.enter_context(tc.tile_pool(name="avp", bufs=1, space="PSUM"))
            po_pool = ctx.enter_context(tc.tile_pool(name="pop", bufs=2, space="PSUM"))
            tmp_pool = ctx.enter_context(tc.tile_pool(name="tmpp", bufs=2))
            rot_pool = ctx.enter_context(tc.tile_pool(name="rotp", bufs=2))
            pr_pool = ctx.enter_context(tc.tile_pool(name="prp", bufs=3))
            edge_pool = ctx.enter_context(tc.tile_pool(name="edgep", bufs=3))
            attn_pool = ctx.enter_context(tc.tile_pool(name="attnp", bufs=2))
            at_pool = ctx.enter_context(tc.tile_pool(name="atp", bufs=2))
            rc_pool = ctx.enter_context(tc.tile_pool(name="rcp", bufs=2))
            ob_pool = ctx.enter_context(tc.tile_pool(name="obp", bufs=2))

            pr_t = {}
            ed_t = {}
            eo_t = {}
            av_t = {}
            attn_t = {}
            at_t = {}
            ob_t = {}
            pp_t = {}

            for s in range(NT + O3):
                a = s  # S0: QKV projection
                b = s - O1  # S1: scores + exp + masks
                c = s - O2  # S2: AV + normalize
                d = s - O3  # S3: WO partial + output

                # ---- S0: QKV matmuls for tile a -> pp (PSUM)
                if a < NT:
                    pp = pp_pool.tile([128, 6, 64], f32, tag="pp")
                    pp_t[a] = pp
                    for kt in range(8):
                        nc.tensor.matmul(
                            pp[:],
                            lhsT=xt_sb[:, a, kt, :],
                            rhs=wq_sb[:, kt, :],
                            start=(kt == 0),
                            stop=(kt == 7),
                        )

                # ---- S1: scores for tile b, chunked by key-block pairs and
                # interleaved with S2/S3 PE work so exp (ACT) gets runway.
                # Chunk 0 holds the diagonal block so its mask runs earliest.
                chunks = []
                if 0 <= b < NT:
                    kb0 = max(0, b - 4)
                    nkb = b - kb0 + 1
                    edge_old = b >= 4
                    if nkb <= 2:
                        chunks = [list(range(nkb))]
                    elif nkb == 3:
                        chunks = [[1, 2], [0]]
                    elif nkb == 4:
                        chunks = [[2, 3], [0, 1]]
                    else:
                        chunks = [[3, 4], [0, 1], [2]]
                    pr = pr_pool.tile([128, 5, 4, 128], bf16, tag="pr")
                    pr_t[b] = pr

                def score_chunk(ci):
                    js = chunks[ci]
                    sct = sc_pool.tile([128, 2, 4, 128], f32, tag="sct", name="sct")
                    for idx, j in enumerate(js):
                        kb = kb0 + j
                        for h in range(4):
                            p0 = (h % 2) * 64
                            slab = h // 2
                            nc.tensor.matmul(
                                sct[:, idx, h, :],
                                lhsT=qk_sb[p0 : p0 + 64, kb, 2, :],
                                rhs=qk_sb[p0 : p0 + 64, b, slab, :],
                                start=True,
                                stop=True,
                            )
                    nc.scalar.activation(
                        pr[:, js[0] : js[0] + len(js), :, :],
                        sct[:, 0 : len(js), :, :],
                        Exp,
                        scale=0.125,
                    )
                    if nkb - 1 in js:
                        ed = edge_pool.tile([128, 4, 128], bf16, tag="ed", name="ed")
                        ed_t[b] = ed
                        nc.gpsimd.tensor_tensor(
                            ed[:],
                            pr[:, nkb - 1, :, :],
                            md_sb[:].broadcast_to((128, 4, 128)),
                            mult,
                        )
                    if edge_old and 0 in js:
                        eo = edge_pool.tile([128, 4, 128], bf16, tag="eo", name="eo")
                        eo_t[b] = eo
                        nc.gpsimd.tensor_tensor(
                            eo[:],
                            pr[:, 0, :, :],
                            mo_sb[:].broadcast_to((128, 4, 128)),
                            mult,
                        )

                if chunks:
                    score_chunk(0)

                # ---- S2 (part 1): AV matmuls for tile c
                if 0 <= c < NT:
                    kb0c = max(0, c - 4)
                    nkbc = c - kb0c + 1
                    edge_old_c = c >= 4
                    av = av_pool.tile([128, 4, 65], f32, tag="av")
                    av_t[c] = av
                    unmasked = [
                        j for j in range(nkbc - 1) if not (j == 0 and edge_old_c)
                    ]
                    masked = ([0] if edge_old_c else []) + [nkbc - 1]
                    order = unmasked + masked
                    for h in range(4):
                        for pos, j in enumerate(order):
                            kb = kb0c + j
                            if j == nkbc - 1:
                                lhs = ed_t[c][:, h, :]
                            elif j == 0 and edge_old_c:
                                lhs = eo_t[c][:, h, :]
                            else:
                                lhs = pr_t[c][:, j, h, :]
                            nc.tensor.matmul(
                                av[:, h, :],
                                lhsT=lhs,
                                rhs=v_sb[:, kb, :],
                                start=(pos == 0),
                                stop=(pos == len(order) - 1),
                            )

                # ---- S1 (cont): second score chunk
                if len(chunks) > 1:
                    score_chunk(1)

                # ---- S3: WO partial projection for tile d
                if 0 <= d < NT:
                    if d % 2 == 0:
                        ob_t[d // 2] = ob_pool.tile(
                            [128, 2, 1024], bf16, tag="ob", name="ob"
                        )
                    ob = ob_t[d // 2]
                    pos_list = []
                    for nb in range(2):
                        po = po_pool.tile([128, 512], f32, tag="po")
                        pos_list.append(po)
                        for kb2 in range(2):
                            nc.tensor.matmul(
                                po[:],
                                lhsT=at_t[d][:, kb2, :],
                                rhs=wo_sb[:, kb2, nb * 512 : (nb + 1) * 512],
                                start=(kb2 == 0),
                                stop=(kb2 == 1),
                            )

                # ---- S1 (cont): third score chunk
                if len(chunks) > 2:
                    score_chunk(2)

                # ---- RoPE for tile a (DVE), k written twice for the scores
                # slab layout, then qk transpose via DMA XBAR on SP
                if a < NT:
                    pp = pp_t[a]
                    a_ap = pp[:, 0:5, 0:HALF]
                    b_ap = pp[:, 0:5, HALF:D_K]
                    co = cos_sb[:, a : a + 1, :].broadcast_to((128, 5, HALF))
                    si = sin_sb[:, a : a + 1, :].broadcast_to((128, 5, HALF))
                    rot6 = rot_pool.tile([128, 6, 64], bf16, tag="rot")
                    t1 = tmp_pool.tile([128, 5, HALF], f32, tag="t1")
                    t2 = tmp_pool.tile([128, 5, HALF], f32, tag="t2")
                    nc.vector.tensor_tensor(t1[:], a_ap, co, mult)
                    nc.vector.tensor_tensor(t2[:], b_ap, si, mult)
                    nc.vector.tensor_tensor(
                        rot6[:, 0:5, 0:HALF], t1[:], t2[:], subtract
                    )
                    nc.vector.tensor_tensor(
                        rot6[:, 5:6, 0:HALF], t1[:, 4:5], t2[:, 4:5], subtract
                    )
                    t3 = tmp_pool.tile([128, 5, HALF], f32, tag="t1")
                    t4 = tmp_pool.tile([128, 5, HALF], f32, tag="t2")
                    nc.vector.tensor_tensor(t3[:], a_ap, si, mult)
                    nc.vector.tensor_tensor(t4[:], b_ap, co, mult)
                    nc.vector.tensor_tensor(rot6[:, 0:5, HALF:D_K], t3[:], t4[:], add)
                    nc.vector.tensor_tensor(
                        rot6[:, 5:6, HALF:D_K], t3[:, 4:5], t4[:, 4:5], add
                    )
                    nc.vector.tensor_copy(v_sb[:, a, 0:64], pp[:, 5, :])
                    nc.sync.dma_start_transpose(qk_sb[:, a, :, :], rot6[:])

                # ---- S2 (part 2): normalize + attn transpose via DMA XBAR
                if 0 <= c < NT:
                    av = av_t[c]
                    rc = rc_pool.tile([128, 4, 1], f32, tag="rc")
                    nc.vector.reciprocal(rc[:], av[:, :, 64:65])
                    attn = attn_pool.tile([128, 4, 64], bf16, tag="attn")
                    attn_t[c] = attn
                    nc.vector.tensor_tensor(
                        attn[:],
                        av[:, :, 0:64],
                        rc[:, :, 0:1].broadcast_to((128, 4, 64)),
                        mult,
                    )
                    at = at_pool.tile([128, 2, 128], bf16, tag="at")
                    at_t[c] = at
                    nc.sync.dma_start_transpose(at[:], attn[:])

                # ---- S3 (cont): drain WO psum (ACT + Pool), output DMA last
                if 0 <= d < NT:
                    ob = ob_t[d // 2]
                    nc.scalar.copy(ob[:, d % 2, 0:512], pos_list[0][:])
                    nc.gpsimd.tensor_copy(ob[:, d % 2, 512:1024], pos_list[1][:])
                    if d % 2 == 1:
                        nc.sync.dma_start(out_d[:, d - 1 : d + 1, :], ob[:])

                # ---- deferred x loads, interleaved behind early transposes
                if s < 4:
                    lo = 6 + s * 3
                    hi = min(NT, lo + 3)
                    if lo < NT:
                        nc.sync.dma_start(xt_sb[:, lo:hi], xt_d[:, lo:hi])

    nc.compile()
    return nc


def _host_inputs(x, WQ, WK, WV, WO, token_positions):
    import ml_dtypes

    perm64 = np.concatenate([np.arange(0, 64, 2), np.arange(1, 64, 2)])
    pos = np.asarray(token_positions).astype(np.float64)
    inv_freq = THETA ** (-np.arange(HALF, dtype=np.float64) / HALF)
    ang = pos[:, None] * inv_freq[None, :]
    cosT = np.ascontiguousarray(
        np.cos(ang).astype(np.float32).reshape(NT, 128, HALF).transpose(1, 0, 2)
    )
    sinT = np.ascontiguousarray(
        np.sin(ang).astype(np.float32).reshape(NT, 128, HALF).transpose(1, 0, 2)
    )

    rk = np.arange(128)[:, None]
    r = np.arange(128)[None, :]
    maskd = (rk <= r).astype(np.float32).astype(ml_dtypes.bfloat16)[:, None, :]
    masko = (rk >= r).astype(np.float32).astype(ml_dtypes.bfloat16)[:, None, :]
    maskd = np.ascontiguousarray(maskd)
    masko = np.ascontiguousarray(masko)

    in_maps = []
    for core in range(8):
        bi, g = core // 4, core % 4
        WQp = (
            WQ[g * 256 : (g + 1) * 256]
            .reshape(GROUP_SIZE, D_K, D_MODEL)[:, perm64, :]
            .reshape(256, D_MODEL)
        )
        WKp = WK[g * 64 : (g + 1) * 64][perm64, :]
        Wf = np.concatenate([WQp, WKp, WV[g * 64 : (g + 1) * 64]], axis=0)
        wqkvT = np.ascontiguousarray(
            Wf.T.reshape(8, 128, 384).transpose(1, 0, 2)
        ).astype(ml_dtypes.bfloat16)
        woT = np.ascontiguousarray(
            WO[:, g * 256 : (g + 1) * 256].T.reshape(2, 128, 1024).transpose(1, 0, 2)
        ).astype(ml_dtypes.bfloat16)
        xT = np.ascontiguousarray(x[bi].T)
        xt4 = np.ascontiguousarray(
            xT.reshape(8, 128, NT, 128).transpose(1, 2, 0, 3)
        ).astype(ml_dtypes.bfloat16)
        in_maps.append(
            {
                "xt": xt4,
                "wqkvT": wqkvT,
                "woT": woT,
                "cosT": cosT,
                "sinT": sinT,
                "maskd": maskd,
                "masko": masko,
            }
        )
    return in_maps


def kernel(x, WQ, WK, WV, WO, token_positions):
    global _PROGRAM
    from concourse.bass_utils import run_bass_kernel_spmd

    x = np.asarray(x, dtype=np.float32)
    WQ = np.asarray(WQ, dtype=np.float32)
    WK = np.asarray(WK, dtype=np.float32)
    WV = np.asarray(WV, dtype=np.float32)
    WO = np.asarray(WO, dtype=np.float32)

    if _PROGRAM is None:
        _PROGRAM = _build_program()
    nc = _PROGRAM

    in_maps = _host_inputs(x, WQ, WK, WV, WO, token_positions)
    res = run_bass_kernel_spmd(nc, in_maps, core_ids=list(range(8)))
    out = np.zeros((B, T, D_MODEL), dtype=np.float32)
    for core in range(8):
        part = np.asarray(res.results[core]["out"], dtype=np.float32)
        out[core // 4] += part.transpose(1, 0, 2).reshape(T, D_MODEL)
    return out


# revision 9
# speedup vs baseline: 1.0829x; 1.0829x over previous
"""GQA with RoPE + sliding-window causal attention on 8 TRN2 NeuronCores.

Sharding: batch (2) x KV-groups (4) -> 8 cores, pure SPMD (no collectives).
Each core computes q/k/v projections for its (batch, group), RoPE, windowed
attention (window=512), and a partial output projection against its group's
WO columns. Host sums the 4 group partials per batch element.

v2: software-pipelined single loop. Per step s, four stages on different
row tiles run concurrently: S0 QKV projection (tile s), S1 scores+exp+mask
(tile s-O1), S2 AV+normalize (tile s-O2), S3 WO partial + output (s-O3).
All matmuls are bf16 (1 cycle/row). The q/k transposes and attn transpose
run on the DMA XBAR (dma_start_transpose), not the PE. x is fully SBUF-
resident (bf16); outputs are written bf16 and summed on host in f32.

qk layout per tile: 3 slabs of 128 partitions x 128 tokens:
  slab0 = [q0|q1] dims, slab1 = [q2|q3], slab2 = [k|k-copy]; scores run
  per-head with 64-partition operands (heads 1,3 at partition base 64
  against the k-copy). Probs accumulate [v|1] so col 64 is the softmax
  denominator. Sliding-window masking multiplies the two edge key blocks
  on GPSIMD off the PE critical path; score chunks are ordered so the
  diagonal (causal) block's exp lands first.
"""

import sys

sys.path.insert(0, "/opt/trn_rl_repo")

import numpy as np
from contextlib import ExitStack

D_MODEL = 1024
GROUP_SIZE = 4
NUM_GROUPS = 4
D_K = 64
THETA = 10000.0
WINDOW = 512
T = 2048
B = 2
NT = T // 128  # 16 row tiles
HALF = D_K // 2

# pipeline offsets: stage S1/S2/S3 of step s handle tiles s-O1/s-O2/s-O3
O1, O2, O3 = 2, 4, 5

_PROGRAM = None


def _build_program():
    from concourse import bacc, tile
    import concourse.mybir as mybir

    f32 = mybir.dt.float32
    bf16 = mybir.dt.bfloat16
    Exp = mybir.ActivationFunctionType.Exp
    mult = mybir.AluOpType.mult
    subtract = mybir.AluOpType.subtract
    add = mybir.AluOpType.add

    nc = bacc.Bacc("TRN2", target_bir_lowering=False, debug=False, num_devices=8)

    xt_d = nc.dram_tensor("xt", [128, NT, 8, 128], bf16, kind="ExternalInput").ap()
    wq_d = nc.dram_tensor("wqkvT", [128, 8, 384], bf16, kind="ExternalInput").ap()
    wo_d = nc.dram_tensor("woT", [128, 2, 1024], bf16, kind="ExternalInput").ap()
    cos_d = nc.dram_tensor("cosT", [128, NT, HALF], f32, kind="ExternalInput").ap()
    sin_d = nc.dram_tensor("sinT", [128, NT, HALF], f32, kind="ExternalInput").ap()
    md_d = nc.dram_tensor("maskd", [128, 1, 128], bf16, kind="ExternalInput").ap()
    mo_d = nc.dram_tensor("masko", [128, 1, 128], bf16, kind="ExternalInput").ap()
    out_d = nc.dram_tensor("out", [128, NT, 1024], bf16, kind="ExternalOutput").ap()

    with tile.TileContext(nc) as tc:
        with ExitStack() as ctx:
            persist = ctx.enter_context(tc.tile_pool(name="persist", bufs=1))
            wq_sb = persist.tile([128, 8, 384], bf16, tag="wq")
            wo_sb = persist.tile([128, 2, 1024], bf16, tag="wo")
            cos_sb = persist.tile([128, NT, HALF], f32, tag="cos")
            sin_sb = persist.tile([128, NT, HALF], f32, tag="sin")
            md_sb = persist.tile([128, 1, 128], bf16, tag="md")
            mo_sb = persist.tile([128, 1, 128], bf16, tag="mo")
            xt_sb = persist.tile([128, NT, 8, 128], bf16, tag="xt")
            qk_sb = persist.tile([128, NT, 3, 128], bf16, tag="qk")
            v_sb = persist.tile([128, NT, 65], bf16, tag="v")

            # startup DMAs: first QKV matmul needs wq + xt tile 0 only
            nc.sync.dma_start(xt_sb[:, 0:1], xt_d[:, 0:1])
            nc.sync.dma_start(wq_sb[:, 0:4], wq_d[:, 0:4])
            nc.sync.dma_start(xt_sb[:, 1:2], xt_d[:, 1:2])
            nc.sync.dma_start(wq_sb[:, 4:8], wq_d[:, 4:8])
            nc.sync.dma_start(cos_sb[:], cos_d[:])
            nc.sync.dma_start(sin_sb[:], sin_d[:])
            nc.sync.dma_start(xt_sb[:, 2:4], xt_d[:, 2:4])
            nc.sync.dma_start(md_sb[:], md_d[:])
            nc.sync.dma_start(mo_sb[:], mo_d[:])
            nc.sync.dma_start(xt_sb[:, 4:6], xt_d[:, 4:6])
            nc.sync.dma_start(xt_sb[:, 6:8], xt_d[:, 6:8])
            nc.sync.dma_start(wo_sb[:], wo_d[:])
            nc.sync.dma_start(xt_sb[:, 8:10], xt_d[:, 8:10])
            nc.sync.dma_start(xt_sb[:, 10:12], xt_d[:, 10:12])
            nc.sync.dma_start(xt_sb[:, 12:14], xt_d[:, 12:14])
            nc.sync.dma_start(xt_sb[:, 14:NT], xt_d[:, 14:NT])
            nc.vector.memset(v_sb[:, :, 64:65], 1.0)

            sc_pool = ctx.enter_context(tc.tile_pool(name="scp", bufs=2, space="PSUM"))
            pp_pool = ctx.enter_context(tc.tile_pool(name="ppp", bufs=1, space="PSUM"))
            av_pool = ctx.enter_context(tc.tile_pool(name="avp", bufs=1, space="PSUM"))
            po_pool = ctx.enter_context(tc.tile_pool(name="pop", bufs=2, space="PSUM"))
            tmp_pool = ctx.enter_context(tc.tile_pool(name="tmpp", bufs=2))
            rot_pool = ctx.enter_context(tc.tile_pool(name="rotp", bufs=2))
            pr_pool = ctx.enter_context(tc.tile_pool(name="prp", bufs=3))
            edge_pool = ctx.enter_context(tc.tile_pool(name="edgep", bufs=3))
            attn_pool = ctx.enter_context(tc.tile_pool(name="attnp", bufs=2))
            at_pool = ctx.enter_context(tc.tile_pool(name="atp", bufs=2))
            rc_pool = ctx.enter_context(tc.tile_pool(name="rcp", bufs=2))
            ob_pool = ctx.enter_context(tc.tile_pool(name="obp", bufs=2))

            pr_t = {}
            ed_t = {}
            eo_t = {}
            av_t = {}
            attn_t = {}
            at_t = {}
            ob_t = {}
            pp_t = {}

            for s in range(NT + O3):
                a = s  # S0: QKV projection
                b = s - O1  # S1: scores + exp + masks
                c = s - O2  # S2: AV + normalize
                d = s - O3  # S3: WO partial + output

                # ---- S0: QKV matmuls for tile a -> pp (PSUM)
                if a < NT:
                    pp = pp_pool.tile([128, 6, 64], f32, tag="pp")
                    pp_t[a] = pp
                    for kt in range(8):
                        nc.tensor.matmul(
                            pp[:],
                            lhsT=xt_sb[:, a, kt, :],
                            rhs=wq_sb[:, kt, :],
                            start=(kt == 0),
                            stop=(kt == 7),
                        )

                # ---- S1: scores for tile b, chunked by key-block pairs and
                # interleaved with S2/S3 PE work so exp (ACT) gets runway.
                # Chunk 0 holds the diagonal block so its mask runs earliest.
                chunks = []
                if 0 <= b < NT:
                    kb0 = max(0, b - 4)
                    nkb = b - kb0 + 1
                    edge_old = b >= 4
                    if nkb <= 2:
                        chunks = [list(range(nkb))]
                    elif nkb == 3:
                        chunks = [[1, 2], [0]]
                    elif nkb == 4:
                        chunks = [[2, 3], [0, 1]]
                    else:
                        chunks = [[3, 4], [0, 1], [2]]
                    pr = pr_pool.tile([128, 5, 4, 128], bf16, tag="pr")
                    pr_t[b] = pr

                def score_chunk(ci):
                    js = chunks[ci]
                    sct = sc_pool.tile([128, 2, 4, 128], f32, tag="sct", name="sct")
                    for idx, j in enumerate(js):
                        kb = kb0 + j
                        for h in range(4):
                            p0 = (h % 2) * 64
                            slab = h // 2
                            nc.tensor.matmul(
                                sct[:, idx, h, :],
                                lhsT=qk_sb[p0 : p0 + 64, kb, 2, :],
                                rhs=qk_sb[p0 : p0 + 64, b, slab, :],
                                start=True,
                                stop=True,
                            )
                    nc.scalar.activation(
                        pr[:, js[0] : js[0] + len(js), :, :],
                        sct[:, 0 : len(js), :, :],
                        Exp,
                        scale=0.125,
                    )
                    if nkb - 1 in js:
                        ed = edge_pool.tile([128, 4, 128], bf16, tag="ed", name="ed")
                        ed_t[b] = ed
                        nc.gpsimd.tensor_tensor(
                            ed[:],
                            pr[:, nkb - 1, :, :],
                            md_sb[:].broadcast_to((128, 4, 128)),
                            mult,
                        )
                    if edge_old and 0 in js:
                        eo = edge_pool.tile([128, 4, 128], bf16, tag="eo", name="eo")
                        eo_t[b] = eo
                        nc.gpsimd.tensor_tensor(
                            eo[:],
                            pr[:, 0, :, :],
                            mo_sb[:].broadcast_to((128, 4, 128)),
                            mult,
                        )

                if chunks:
                    score_chunk(0)

                # ---- S2 (part 1): AV matmuls for tile c
                if 0 <= c < NT:
                    kb0c = max(0, c - 4)
                    nkbc = c - kb0c + 1
                    edge_old_c = c >= 4
                    av = av_pool.tile([128, 4, 65], f32, tag="av")
                    av_t[c] = av
                    unmasked = [
                        j for j in range(nkbc - 1) if not (j == 0 and edge_old_c)
                    ]
                    masked = ([0] if edge_old_c else []) + [nkbc - 1]
                    order = unmasked + masked
                    for h in range(4):
                        for pos, j in enumerate(order):
                            kb = kb0c + j
                            if j == nkbc - 1:
                                lhs = ed_t[c][:, h, :]
                            elif j == 0 and edge_old_c:
                                lhs = eo_t[c][:, h, :]
                            else:
                                lhs = pr_t[c][:, j, h, :]
                            nc.tensor.matmul(
                                av[:, h, :],
                                lhsT=lhs,
                                rhs=v_sb[:, kb, :],
                                start=(pos == 0),
                                stop=(pos == len(order) - 1),
                            )

                # ---- S1 (cont): second score chunk
                if len(chunks) > 1:
                    score_chunk(1)

                # ---- S3: WO partial projection for tile d
                if 0 <= d < NT:
                    if d % 2 == 0:
                        ob_t[d // 2] = ob_pool.tile(
                            [128, 2, 1024], bf16, tag="ob", name="ob"
                        )
                    ob = ob_t[d // 2]
                    pos_list = []
                    for nb in range(2):
                        po = po_pool.tile([128, 512], f32, tag="po")
                        pos_list.append(po)
                        for kb2 in range(2):
                            nc.tensor.matmul(
                                po[:],
                                lhsT=at_t[d][:, kb2, :],
                                rhs=wo_sb[:, kb2, nb * 512 : (nb + 1) * 512],
                                start=(kb2 == 0),
                                stop=(kb2 == 1),
                            )

                # ---- S1 (cont): third score chunk
                if len(chunks) > 2:
                    score_chunk(2)

                # ---- RoPE for tile a (DVE), k written twice for the scores
                # slab layout, then qk transpose via DMA XBAR on SP
                if a < NT:
                    pp = pp_t[a]
                    a_ap = pp[:, 0:5, 0:HALF]
                    b_ap = pp[:, 0:5, HALF:D_K]
                    co = cos_sb[:, a : a + 1, :].broadcast_to((128, 5, HALF))
                    si = sin_sb[:, a : a + 1, :].broadcast_to((128, 5, HALF))
                    rot6 = rot_pool.tile([128, 6, 64], bf16, tag="rot")
                    t1 = tmp_pool.tile([128, 5, HALF], f32, tag="t1")
                    t2 = tmp_pool.tile([128, 5, HALF], f32, tag="t2")
                    nc.vector.tensor_tensor(t1[:], a_ap, co, mult)
                    nc.vector.tensor_tensor(t2[:], b_ap, si, mult)
                    nc.vector.tensor_tensor(
                        rot6[:, 0:5, 0:HALF], t1[:], t2[:], subtract
                    )
                    nc.vector.tensor_tensor(
                        rot6[:, 5:6, 0:HALF], t1[:, 4:5], t2[:, 4:5], subtract
                    )
                    t3 = tmp_pool.tile([128, 5, HALF], f32, tag="t1")
                    t4 = tmp_pool.tile([128, 5, HALF], f32, tag="t2")
                    nc.vector.tensor_tensor(t3[:], a_ap, si, mult)
                    nc.vector.tensor_tensor(t4[:], b_ap, co, mult)
                    nc.vector.tensor_tensor(rot6[:, 0:5, HALF:D_K], t3[:], t4[:], add)
                    nc.vector.tensor_tensor(
                        rot6[:, 5:6, HALF:D_K], t3[:, 4:5], t4[:, 4:5], add
                    )
                    nc.vector.tensor_copy(v_sb[:, a, 0:64], pp[:, 5, :])
                    nc.sync.dma_start_transpose(qk_sb[:, a, :, :], rot6[:])

                # ---- S2 (part 2): normalize + attn transpose via DMA XBAR
                if 0 <= c < NT:
                    av = av_t[c]
                    rc = rc_pool.tile([128, 4, 1], f32, tag="rc")
                    nc.vector.reciprocal(rc[:], av[:, :, 64:65])
                    attn = attn_pool.tile([128, 4, 64], bf16, tag="attn")
                    attn_t[c] = attn
                    nc.vector.tensor_tensor(
                        attn[:],
                        av[:, :, 0:64],
                        rc[:, :, 0:1].broadcast_to((128, 4, 64)),
                        mult,
                    )
                    at = at_pool.tile([128, 2, 128], bf16, tag="at")
                    at_t[c] = at
                    nc.sync.dma_start_transpose(at[:], attn[:])

                # ---- S3 (cont): drain WO psum (ACT + Pool), output DMA last
                if 0 <= d < NT:
                    ob = ob_t[d // 2]
                    nc.scalar.copy(ob[:, d % 2, 0:512], pos_list[0][:])
                    nc.gpsimd.tensor_copy(ob[:, d % 2, 512:1024], pos_list[1][:])
                    if d % 2 == 1:
                        nc.sync.dma_start(out_d[:, d - 1 : d + 1, :], ob[:])


    nc.compile()
    return nc


def _host_inputs(x, WQ, WK, WV, WO, token_positions):
    import ml_dtypes

    perm64 = np.concatenate([np.arange(0, 64, 2), np.arange(1, 64, 2)])
    pos = np.asarray(token_positions).astype(np.float64)
    inv_freq = THETA ** (-np.arange(HALF, dtype=np.float64) / HALF)
    ang = pos[:, None] * inv_freq[None, :]
    cosT = np.ascontiguousarray(
        np.cos(ang).astype(np.float32).reshape(NT, 128, HALF).transpose(1, 0, 2)
    )
    sinT = np.ascontiguousarray(
        np.sin(ang).astype(np.float32).reshape(NT, 128, HALF).transpose(1, 0, 2)
    )

    rk = np.arange(128)[:, None]
    r = np.arange(128)[None, :]
    maskd = (rk <= r).astype(np.float32).astype(ml_dtypes.bfloat16)[:, None, :]
    masko = (rk >= r).astype(np.float32).astype(ml_dtypes.bfloat16)[:, None, :]
    maskd = np.ascontiguousarray(maskd)
    masko = np.ascontiguousarray(masko)

    in_maps = []
    for core in range(8):
        bi, g = core // 4, core % 4
        WQp = (
            WQ[g * 256 : (g + 1) * 256]
            .reshape(GROUP_SIZE, D_K, D_MODEL)[:, perm64, :]
            .reshape(256, D_MODEL)
        )
        WKp = WK[g * 64 : (g + 1) * 64][perm64, :]
        Wf = np.concatenate([WQp, WKp, WV[g * 64 : (g + 1) * 64]], axis=0)
        wqkvT = np.ascontiguousarray(
            Wf.T.reshape(8, 128, 384).transpose(1, 0, 2)
        ).astype(ml_dtypes.bfloat16)
        woT = np.ascontiguousarray(
            WO[:, g * 256 : (g + 1) * 256].T.reshape(2, 128, 1024).transpose(1, 0, 2)
        ).astype(ml_dtypes.bfloat16)
        xT = np.ascontiguousarray(x[bi].T)
        xt4 = np.ascontiguousarray(
            xT.reshape(8, 128, NT, 128).transpose(1, 2, 0, 3)
        ).astype(ml_dtypes.bfloat16)
        in_maps.append(
            {
                "xt": xt4,
                "wqkvT": wqkvT,
                "woT": woT,
                "cosT": cosT,
                "sinT": sinT,
                "maskd": maskd,
                "masko": masko,
            }
        )
    return in_maps


def kernel(x, WQ, WK, WV, WO, token_positions):
    global _PROGRAM
    from concourse.bass_utils import run_bass_kernel_spmd

    x = np.asarray(x, dtype=np.float32)
    WQ = np.asarray(WQ, dtype=np.float32)
    WK = np.asarray(WK, dtype=np.float32)
    WV = np.asarray(WV, dtype=np.float32)
    WO = np.asarray(WO, dtype=np.float32)

    if _PROGRAM is None:
        _PROGRAM = _build_program()
    nc = _PROGRAM

    in_maps = _host_inputs(x, WQ, WK, WV, WO, token_positions)
    res = run_bass_kernel_spmd(nc, in_maps, core_ids=list(range(8)))
    out = np.zeros((B, T, D_MODEL), dtype=np.float32)
    for core in range(8):
        part = np.asarray(res.results[core]["out"], dtype=np.float32)
        out[core // 4] += part.transpose(1, 0, 2).reshape(T, D_MODEL)
    return out
